# revision 20
# baseline (speedup 1.0000x reference)
"""ArcMargin head (ArcFace) distributed over 8 TRN2 NeuronCores.

Strategy (classification / tensor parallel):
  - weight [C, D] is sharded along C across the 8 cores (12500 classes each,
    zero-padded to 12544 = 98*128 for uniform tiling); embeddings + per-row
    gathered weight rows (weight[labels]) are replicated.
  - Key algebraic fact: the ArcFace margin (phi) only replaces the ONE target
    element per row; every other element of the [B, C] output is just
    SCALE * cosine.  So the device computes out = (SCALE*e_hat) @ w_hat.T with
    a bf16 TensorEngine matmul (f32 accumulate), and separately computes the
    2048 target values phi from the replicated weight[labels] rows, scattering
    them into the output with per-row-chunk indirect DMAs (rows whose label
    falls outside a core's shard use an out-of-bounds offset that the DMA's
    bounds check silently skips).
  - Normalization (x / max(||x||, eps)), the *64 logit scale and the f32->bf16
    cast are fused into the per-tile load pipeline; the weight shard is
    transposed on-chip with TensorEngine transpose ops so the big matmul can
    stream [D, C]-major bf16 tiles.
  - The output is split into 16 per-row-chunk DRAM tensors so each scatter
    only depends on its own chunk's column-block writes (short kernel tail).
"""

import math
import sys

import numpy as np

for _p in ("/opt/trn_rl_repo",):
    if _p not in sys.path:
        sys.path.append(_p)

import concourse.bass as bass
import concourse.tile as tile
from concourse import bacc
from concourse import mybir
from concourse.bass_utils import run_bass_kernel_spmd
from concourse.masks import make_identity

SCALE = 64.0
MARGIN = 0.5
COS_M = math.cos(MARGIN)
SIN_M = math.sin(MARGIN)
TH = math.cos(math.pi - MARGIN)
MM = math.sin(math.pi - MARGIN) * MARGIN

B, D, C = 2048, 512, 100000
N_CORES = 8
CS = C // N_CORES          # 12500 real classes per core
CSP = 12544                # padded classes per core (98 * 128)
OOB = 1 << 30              # scatter offset sentinel for "not my row"

F32 = mybir.dt.float32
BF16 = mybir.dt.bfloat16
I32 = mybir.dt.int32
AF = mybir.ActivationFunctionType
ALU = mybir.AluOpType


def build_program(b=B, d=D, csp=CSP):
    """Build the (SPMD-uniform) single-core Bass program."""
    mb = b // 128          # batch row-chunks
    kc = d // 128          # contraction chunks
    nc = bacc.Bacc()

    emb_d = nc.declare_dram_parameter("emb", [b, d], F32, isOutput=False)
    wsh_d = nc.declare_dram_parameter("wsh", [csp, d], F32, isOutput=False)
    soff_d = nc.declare_dram_parameter("soff", [128, mb], I32, isOutput=False)
    goff_d = nc.declare_dram_parameter("goff", [128, mb], I32, isOutput=False)
    ident_d = nc.declare_dram_parameter("ident", [128, 128], F32, isOutput=False)
    # one flat output tensor per batch row-chunk: scatters then only wait on
    # their own chunk's writes, and offsets are plain element indices
    out_ds = [
        nc.declare_dram_parameter(f"out{m}", [128 * csp, 1], F32, isOutput=True)
        for m in range(mb)
    ]

    # column blocks of up to 2048; the small remainder block goes FIRST so
    # the prologue is short and the final block keeps the PE dense
    ctbs = []
    c0 = 0
    while c0 < csp:
        w = min(2048, csp - c0)
        ctbs.append((c0, w))
        c0 += w
    ctbs.sort(key=lambda t: t[1])  # remainder (smallest) first
    max_cbw = max(w for _, w in ctbs)

    with tile.TileContext(nc) as tc:
        with (
            tc.tile_pool(name="const", bufs=1) as constp,
            tc.tile_pool(name="persist", bufs=1) as persist,
            tc.tile_pool(name="ld", bufs=3) as ldp,
            tc.tile_pool(name="bf", bufs=3) as bfp,
            tc.tile_pool(name="scr", bufs=2) as scrp,
            tc.tile_pool(name="small", bufs=4) as smp,
            tc.tile_pool(name="wtb", bufs=3) as wtbp,
            tc.tile_pool(name="outp", bufs=3) as outp,
            tc.tile_pool(name="tpsum", bufs=2, space="PSUM") as tpsum,
            tc.tile_pool(name="cpsum", bufs=3, space="PSUM") as cpsum,
        ):
            ident = constp.tile([128, 128], BF16)
            nc.gpsimd.dma_start(out=ident[:], in_=ident_d[:])  # SWDGE casts f32->bf16
            zb = constp.tile([128, 1], F32, tag="zb")
            nc.vector.memset(zb[:], 0.0)
            epsb = constp.tile([128, 1], F32, tag="epsb")
            nc.vector.memset(epsb[:], 1e-24)
            s2b = constp.tile([128, 1], F32, tag="s2b")
            nc.vector.memset(s2b[:], SCALE * SCALE)

            eT = persist.tile([128, kc, b], BF16)      # (64*e_hat)^T
            ebf_all = persist.tile([128, mb, d], BF16)  # 64*e_hat, natural layout
            svec = persist.tile([128, mb], F32)        # 64*cos(target)
            sofft = persist.tile([128, mb], I32)       # scatter element offsets
            gofft = persist.tile([128, mb], I32)       # gather row offsets
            tval = persist.tile([128, mb], F32)        # 64*phi / else-branch

            def rownorm_recip(x_f32, tag):
                # [128,1] f32 = 1 / max(||x_row||, ~1e-12)
                sq = scrp.tile([128, d], BF16, tag="sq_scr")
                ssq = smp.tile([128, 1], F32, tag=f"{tag}_ssq")
                nc.scalar.activation(
                    out=sq[:], in_=x_f32[:], func=AF.Square, bias=zb[:],
                    accum_out=ssq[:],
                )
                nrm = smp.tile([128, 1], F32, tag=f"{tag}_nrm")
                nc.scalar.activation(out=nrm[:], in_=ssq[:], func=AF.Sqrt, bias=epsb[:])
                rec = smp.tile([128, 1], F32, tag=f"{tag}_rec")
                nc.vector.reciprocal(out=rec[:], in_=nrm[:])
                return rec

            # ---------- Phase A + B(0) prologue (interleaved loads) ----------
            nc.sync.dma_start(out=sofft[:], in_=soff_d[:])
            nc.sync.dma_start(out=gofft[:], in_=goff_d[:])
            out2ds = [
                od[:].rearrange("(a b) o -> a (b o)", b=csp) for od in out_ds
            ]
            evac_flip = [0]
            EGRP = 2          # emb row-chunks per DMA (512 KB)
            WGRP = 4          # weight row-chunks per DMA (1 MB)

            def emb_unit(m0):
                eg = ldp.tile([128, EGRP, d], F32, tag="e_ld", name=f"eg_{m0}")
                nc.sync.dma_start(
                    out=eg[:],
                    in_=emb_d[m0 * 128:(m0 + EGRP) * 128, :].rearrange(
                        "(g p) d -> p g d", p=128
                    ),
                )
                for g in range(EGRP):
                    m = m0 + g
                    et = eg[:, g, :]
                    rec = rownorm_recip(et, "e")
                    rec64 = smp.tile([128, 1], F32, tag="e_rec64")
                    nc.scalar.mul(out=rec64[:], in_=rec[:], mul=SCALE)
                    ebf = ebf_all[:, m, :]
                    nc.vector.tensor_scalar_mul(out=ebf, in0=et, scalar1=rec64[:])
                    for k in range(kc):
                        pt = tpsum.tile([128, 128], BF16)
                        nc.tensor.transpose(
                            out=pt[:], in_=ebf[:, k * 128:(k + 1) * 128],
                            identity=ident[:],
                        )
                        nc.vector.tensor_copy(
                            out=eT[:, k, m * 128:(m + 1) * 128], in_=pt[:]
                        )

            def b_alloc(cb_idx):
                wtb = wtbp.tile(
                    [128, kc, max_cbw], BF16, tag="wtb", name=f"wtb_{cb_idx}"
                )
                wnb_all = wtbp.tile(
                    [128, max_cbw // 128, d], BF16, tag="wnb_all",
                    name=f"wnba_{cb_idx}", bufs=2,
                )
                return wtb, wnb_all

            def b_load_norm(cb_idx, wnb_all, cc, state):
                # one 128-row weight chunk: (load every WGRP), norm -> wnb_all
                cb0, cbw = ctbs[cb_idx]
                ncc = cbw // 128
                if cc % WGRP == 0:
                    ng = min(WGRP, ncc - cc)
                    state["wg"] = ldp.tile(
                        [128, WGRP, d], F32, tag="w_ld", name=f"wg_{cb_idx}_{cc}"
                    )
                    nc.sync.dma_start(
                        out=state["wg"][:, :ng, :],
                        in_=wsh_d[
                            cb0 + cc * 128: cb0 + (cc + ng) * 128, :
                        ].rearrange("(g p) d -> p g d", p=128),
                    )
                wld = state["wg"][:, cc % WGRP, :]
                wrec = rownorm_recip(wld, "w")
                nc.vector.tensor_scalar_mul(
                    out=wnb_all[:, cc, :], in0=wld, scalar1=wrec[:]
                )

            def b_transpose(wnb_all, wtb, cc):
                for k in range(kc):
                    pt = tpsum.tile([128, 128], BF16)
                    nc.tensor.transpose(
                        out=pt[:], in_=wnb_all[:, cc, k * 128:(k + 1) * 128],
                        identity=ident[:],
                    )
                    nc.vector.tensor_copy(
                        out=wtb[:, k, cc * 128:(cc + 1) * 128], in_=pt[:]
                    )

            def phase_wsel(m):
                # gather this chunk's owned target weight rows from the shard
                wt = ldp.tile([128, d], F32, tag="ws_ld", name=f"ws_{m}")
                nc.gpsimd.indirect_dma_start(
                    out=wt[:],
                    out_offset=None,
                    in_=wsh_d[:],
                    in_offset=bass.IndirectOffsetOnAxis(
                        ap=gofft[:, m:m + 1], axis=0
                    ),
                    bounds_check=csp - 1,
                    oob_is_err=False,
                )
                wrec = rownorm_recip(wt, "ws")
                wnb = bfp.tile([128, d], BF16, tag="ws_bf")
                nc.vector.tensor_scalar_mul(out=wnb[:], in0=wt[:], scalar1=wrec[:])
                ttr_scr = scrp.tile([128, d], BF16, tag="ttr_scr")
                nc.vector.tensor_tensor(
                    out=ttr_scr[:], in0=ebf_all[:, m, :], in1=wnb[:], op=ALU.mult
                )
                nc.vector.tensor_reduce(
                    out=svec[:, m:m + 1], in_=ttr_scr[:],
                    axis=mybir.AxisListType.X, op=ALU.add,
                )

            def scatter(m):
                nc.gpsimd.indirect_dma_start(
                    out=out_ds[m][:],
                    out_offset=bass.IndirectOffsetOnAxis(
                        ap=sofft[:, m:m + 1], axis=0
                    ),
                    in_=tval[:, m:m + 1],
                    in_offset=None,
                    bounds_check=128 * csp - 1,
                    oob_is_err=False,
                )

            # prologue: interleave embedding loads with B(0)+B(1) prep
            wtb0, wnba0 = b_alloc(0)
            multi = len(ctbs) > 1
            wtb1, wnba1 = b_alloc(1) if multi else (None, None)
            st0 = {}
            st1 = {}
            ncc0 = ctbs[0][1] // 128
            ncc1 = ctbs[1][1] // 128 if multi else 0
            eidx = 0
            for cc in range(ncc0):
                b_load_norm(0, wnba0, cc, st0)
            cc1 = 0
            while eidx < mb or cc1 < ncc1:
                if eidx < mb:
                    emb_unit(eidx)
                    eidx += EGRP
                for _ in range(2):
                    if cc1 < ncc1:
                        b_load_norm(1, wnba1, cc1, st1)
                        cc1 += 1
            for cc in range(ncc0):
                b_transpose(wnba0, wtb0, cc)

            # ---------- main loop: C(cb) with B(cb+1) interleaved 1:1 ----------
            wsel_done = 0
            phi_emitted = [False]

            def phi_block():
                phi_emitted[0] = True
                s2 = smp.tile([128, mb], F32, tag="s2")
                nc.scalar.activation(
                    out=s2[:], in_=svec[:], func=AF.Square, bias=zb[:]
                )
                rl = smp.tile([128, mb], F32, tag="rl")
                nc.scalar.activation(
                    out=rl[:], in_=s2[:], func=AF.Relu, bias=s2b[:], scale=-1.0
                )
                sn = smp.tile([128, mb], F32, tag="sn")
                nc.scalar.activation(out=sn[:], in_=rl[:], func=AF.Sqrt, bias=zb[:])
                pc = smp.tile([128, mb], F32, tag="pc")
                nc.vector.tensor_scalar_mul(out=pc[:], in0=svec[:], scalar1=COS_M)
                smt = smp.tile([128, mb], F32, tag="smt")
                nc.vector.tensor_scalar_mul(out=smt[:], in0=sn[:], scalar1=SIN_M)
                ph = smp.tile([128, mb], F32, tag="ph")
                nc.vector.tensor_tensor(
                    out=ph[:], in0=pc[:], in1=smt[:], op=ALU.subtract
                )
                eb = smp.tile([128, mb], F32, tag="eb")
                nc.vector.tensor_scalar_add(
                    out=eb[:], in0=svec[:], scalar1=-SCALE * MM
                )
                mk = smp.tile([128, mb], mybir.dt.uint8, tag="mk")
                nc.vector.tensor_scalar(
                    out=mk[:], in0=svec[:], scalar1=SCALE * TH, scalar2=None,
                    op0=ALU.is_gt,
                )
                nc.vector.select(out=tval[:], mask=mk[:], on_true=ph[:], on_false=eb[:])

            cur_wtb = wtb0
            cur_wnba = wnba0
            for cb_idx, (cb0, cbw) in enumerate(ctbs):
                last_cb = cb_idx == len(ctbs) - 1
                nxt_wtb = nxt_wnba = None
                nxt_state = {}
                nxt_ncc = 0
                if cb_idx == 0:
                    nxt_wtb, nxt_wnba = wtb1, wnba1
                    nxt_ncc = ncc1
                elif not last_cb:
                    nxt_wtb, nxt_wnba = b_alloc(cb_idx + 1)
                    nxt_ncc = ctbs[cb_idx + 1][1] // 128

                nps = (cbw + 1023) // 1024   # 1024-wide psum tiles (2 banks)
                for m in range(mb):
                    pss = [
                        cpsum.tile([128, 1024], F32, tag="mmps", name=f"mmps_{j}")
                        for j in range(nps)
                    ]
                    for k in range(kc):
                        for j5 in range((cbw + 511) // 512):
                            s0 = j5 * 512
                            sw = min(512, cbw - s0)
                            nc.tensor.matmul(
                                out=pss[j5 // 2][:, (j5 % 2) * 512:(j5 % 2) * 512 + sw],
                                lhsT=eT[:, k, m * 128:(m + 1) * 128],
                                rhs=cur_wtb[:, k, s0:s0 + sw],
                                start=(k == 0),
                                stop=(k == kc - 1),
                            )
                    ot = outp.tile([128, max_cbw], F32, tag="o_t")
                    for j in range(nps):
                        s0 = j * 1024
                        sw = min(1024, cbw - s0)
                        if evac_flip[0] % 2 == 0:
                            nc.vector.tensor_copy(
                                out=ot[:, s0:s0 + sw], in_=pss[j][:, :sw]
                            )
                        else:
                            nc.scalar.copy(out=ot[:, s0:s0 + sw], in_=pss[j][:, :sw])
                        evac_flip[0] += 1
                    nc.sync.dma_start(
                        out=out2ds[m][:, cb0:cb0 + cbw], in_=ot[:, :cbw]
                    )
                    # next block: loads+norms early, PE transposes late
                    if not last_cb:
                        if m < 4 and cb_idx > 0:
                            for cc in range(m * WGRP, min((m + 1) * WGRP, nxt_ncc)):
                                b_load_norm(cb_idx + 1, nxt_wnba, cc, nxt_state)
                        elif m >= 8 and (m - 8) * 2 < nxt_ncc:
                            for cc in range((m - 8) * 2, min((m - 8) * 2 + 2, nxt_ncc)):
                                b_transpose(nxt_wnba, nxt_wtb, cc)
                    # sprinkle wsel row-dot chunks into the first blocks
                    if cb_idx >= 1 and wsel_done < mb and m % 4 == 3:
                        phase_wsel(wsel_done)
                        wsel_done += 1
                    if last_cb:
                        if m == 0:
                            while wsel_done < mb:
                                phase_wsel(wsel_done)
                                wsel_done += 1
                            phi_block()
                        scatter(m)
                cur_wtb = nxt_wtb
                cur_wnba = nxt_wnba

    nc.compile()
    return nc


_CACHE = {}


def _get_program():
    if "nc" not in _CACHE:
        _CACHE["nc"] = build_program()
    return _CACHE["nc"]


def make_in_maps(embeddings, labels, weight):
    embeddings = np.ascontiguousarray(np.asarray(embeddings, dtype=np.float32))
    weight = np.asarray(weight, dtype=np.float32)
    labels_np = np.asarray(labels).astype(np.int64)
    ident = np.eye(128, dtype=np.float32)
    in_maps = []
    for k in range(N_CORES):
        wsh = np.zeros((CSP, D), np.float32)
        wsh[:CS] = weight[k * CS:(k + 1) * CS]
        own = (labels_np // CS) == k
        col = labels_np - k * CS
        # offset within the row-chunk's flat [128*CSP] tensor: p*CSP + col
        p = np.arange(B) % 128
        soff = np.where(own, p * CSP + col, OOB).astype(np.int64)
        soff_arr = np.ascontiguousarray(
            soff.reshape(B // 128, 128).T.astype(np.int32)
        )
        goff = np.where(own, col, OOB).astype(np.int64)
        goff_arr = np.ascontiguousarray(
            goff.reshape(B // 128, 128).T.astype(np.int32)
        )
        in_maps.append(
            {"emb": embeddings, "wsh": wsh, "soff": soff_arr,
             "goff": goff_arr, "ident": ident}
        )
    return in_maps


def _gather(results):
    full = np.empty((B, C), np.float32)
    for k in range(N_CORES):
        for m in range(B // 128):
            shard = np.asarray(results[k][f"out{m}"]).reshape(128, CSP)
            full[m * 128:(m + 1) * 128, k * CS:(k + 1) * CS] = shard[:, :CS]
    return full


def kernel(embeddings, labels, weight):
    nc = _get_program()
    in_maps = make_in_maps(embeddings, labels, weight)
    res = run_bass_kernel_spmd(nc, in_maps, core_ids=list(range(N_CORES)))
    return _gather(res.results)


def kernel_profiled(embeddings, labels, weight, **kw):
    """Like kernel() but also returns the BassKernelResults (exec_time_ns)."""
    nc = _get_program()
    in_maps = make_in_maps(embeddings, labels, weight)
    res = run_bass_kernel_spmd(
        nc, in_maps, core_ids=list(range(N_CORES)), trace=True, **kw
    )
    return _gather(res.results), res
